# revision 1
# baseline (speedup 1.0000x reference)
"""Trainium2 Bass kernel for single-query attention over per-sample concepts.

    sab[b, k] = (query[b] . concept[b, k]) / sqrt(D)
    score     = softmax(sab, axis=-1)
    out[b]    = sum_k score[b, k] * concept[b, k]

Shapes: query [256, 1024] f32, concept [256, 2048, 1024] f32 -> out [256, 1024].

Sharding: pure data parallel, batch 256 split as 32 samples on each of 8
NeuronCores. Memory-bound: each core streams its 256 MiB concept shard once.

Per-core dataflow, per sample b (all tiles [128 k-partitions, 1024 d-free]):
  - qb = broadcast(query[b]) to 128 partitions (GPSIMD partition_broadcast)
  - DMA c-tile t (128 k's), alternating the SP/ACT HWDGE rings (dual-ring
    issue lifts sustained HBM bandwidth ~370 -> ~395 GB/s)
  - DVE scalar_tensor_tensor: elementwise (c*scale)*qb with accum_out
    -> raw scores s[128, 1] per tile (fused multiply+reduce, one pass)
  - ACT exp per tile -> e[128, 1]
  - PE matmul: acc[1, 0:512] += e_t.T @ c_t[:, 0:512], same for 512:1024
    (fp32, PSUM accumulate over the 16 k-tiles)
  - denominator: ACT copy of e-columns with accum_out -> per-partition sums,
    then PE matmul with ones stationary -> [1, 1] in PSUM
  - DVE reciprocal, ACT Copy-with-scale to normalize, DMA out row.
"""

import numpy as np
from contextlib import ExitStack

import concourse.bacc as bacc
import concourse.tile as tile
from concourse import mybir
from concourse.bass_utils import run_bass_kernel_spmd

B, K, D = 256, 2048, 1024
NCORES = 8
BL = B // NCORES          # 32 samples per core
KT = 128                  # k-tile size (partition dim)
NT = K // KT              # 16 k-tiles per sample
SCALE = 1.0 / float(np.sqrt(D))

_cache = {}


def build_nc():
    nc = bacc.Bacc("TRN2", target_bir_lowering=False, debug=False,
                   num_devices=NCORES)
    q = nc.dram_tensor("query", [BL, D], mybir.dt.float32, kind="ExternalInput")
    c = nc.dram_tensor("concept", [BL, K, D], mybir.dt.float32r,
                       kind="ExternalInput")
    out = nc.dram_tensor("out", [BL, D], mybir.dt.float32,
                         kind="ExternalOutput")
    f32 = mybir.dt.float32

    f32r = mybir.dt.float32r

    with tile.TileContext(nc) as tc, ExitStack() as ctx:
        cpool = ctx.enter_context(tc.tile_pool(name="c", bufs=16))
        qpool = ctx.enter_context(tc.tile_pool(name="q", bufs=3))
        spool = ctx.enter_context(tc.tile_pool(name="scr", bufs=2))
        epool = ctx.enter_context(tc.tile_pool(name="e", bufs=3))
        onepool = ctx.enter_context(tc.tile_pool(name="one", bufs=1))
        opool = ctx.enter_context(tc.tile_pool(name="o", bufs=4))
        ppool = ctx.enter_context(tc.tile_pool(name="ps", bufs=2, space="PSUM"))
        dpool = ctx.enter_context(tc.tile_pool(name="dn", bufs=2, space="PSUM"))

        ones = onepool.tile([KT, 1], f32)
        nc.vector.memset(ones[:], 1.0)

        for b in range(BL):
            qrow = qpool.tile([1, D], f32)
            nc.scalar.dma_start(out=qrow[:], in_=q[b : b + 1, :])
            qb = qpool.tile([KT, D], f32)
            nc.gpsimd.partition_broadcast(qb[:], qrow[:])

            scols = epool.tile([KT, NT], f32)
            ecols = epool.tile([KT, NT], f32r)
            acc_lo = ppool.tile([1, 512], f32)
            acc_hi = ppool.tile([1, 512], f32)

            for t in range(NT):
                ct = cpool.tile([KT, D], f32r)
                # alternate the two HWDGE rings (SP / ACT) for issue overlap
                dma_eng = nc.sync if t % 2 == 0 else nc.scalar
                dma_eng.dma_start(out=ct[:], in_=c[b, t * KT : (t + 1) * KT, :])
                scr = spool.tile([KT, D], f32)
                nc.vector.scalar_tensor_tensor(
                    out=scr[:],
                    in0=ct[:].bitcast(f32),
                    scalar=SCALE,
                    in1=qb[:],
                    op0=mybir.AluOpType.mult,
                    op1=mybir.AluOpType.mult,
                    accum_out=scols[:, t : t + 1],
                )
                nc.scalar.activation(
                    out=ecols[:, t : t + 1],
                    in_=scols[:, t : t + 1],
                    func=mybir.ActivationFunctionType.Exp,
                )
                e_t = ecols[:, t : t + 1]
                nc.tensor.matmul(acc_lo[:], e_t, ct[:, 0:512],
                                 start=(t == 0), stop=(t == NT - 1))
                nc.tensor.matmul(acc_hi[:], e_t, ct[:, 512:1024],
                                 start=(t == 0), stop=(t == NT - 1))

            # denominator: per-partition sums of e, then reduce across
            # partitions with a ones-stationary matmul
            ered = epool.tile([KT, 1], f32)
            escr = spool.tile([KT, NT], f32)
            nc.scalar.activation(
                out=escr[:],
                in_=ecols[:].bitcast(f32),
                func=mybir.ActivationFunctionType.Copy,
                accum_out=ered[:],
            )
            denom = dpool.tile([1, 1], f32)
            nc.tensor.matmul(denom[:], ones[:], ered[:], start=True, stop=True)

            recip = opool.tile([1, 1], f32)
            nc.vector.reciprocal(recip[:], denom[:])

            orow = opool.tile([1, D], f32)
            nc.scalar.activation(out=orow[:, 0:512], in_=acc_lo[:],
                                 func=mybir.ActivationFunctionType.Copy,
                                 scale=recip[:])
            nc.scalar.activation(out=orow[:, 512:1024], in_=acc_hi[:],
                                 func=mybir.ActivationFunctionType.Copy,
                                 scale=recip[:])
            nc.scalar.dma_start(out=out[b : b + 1, :], in_=orow[:])

    nc.compile()
    return nc


def _run(query, concept, trace=False, trace_kwargs=None):
    if "nc" not in _cache:
        _cache["nc"] = build_nc()
    nc = _cache["nc"]
    in_maps = []
    for i in range(NCORES):
        in_maps.append({
            "query": np.ascontiguousarray(query[i * BL : (i + 1) * BL]),
            "concept": np.ascontiguousarray(concept[i * BL : (i + 1) * BL]),
        })
    res = run_bass_kernel_spmd(
        nc, in_maps, core_ids=list(range(NCORES)),
        trace=trace, **(trace_kwargs or {}),
    )
    out = np.concatenate([res.results[i]["out"] for i in range(NCORES)], axis=0)
    return out.astype(np.float32), res


def kernel(query: np.ndarray, concept: np.ndarray) -> np.ndarray:
    out, _ = _run(np.asarray(query, np.float32), np.asarray(concept, np.float32))
    return out



# revision 10
# speedup vs baseline: 1.0402x; 1.0402x over previous
"""Trainium2 Bass kernel for single-query attention over per-sample concepts.

    sab[b, k] = (query[b] . concept[b, k]) / sqrt(D)
    score     = softmax(sab, axis=-1)
    out[b]    = sum_k score[b, k] * concept[b, k]

Shapes: query [256, 1024] f32, concept [256, 2048, 1024] f32 -> out [256, 1024].

Sharding: pure data parallel, batch 256 split as 32 samples on each of 8
NeuronCores. Memory-bound: each core streams its 256 MiB concept shard once.

v2 dataflow (vs the 512x512KiB-DMA baseline):
  - concept viewed as [32 samples, 2 granules, 128 partitions, 8192 floats]:
    one dma_start per 4 MiB granule (32 KiB contiguous per partition line,
    64 big DMAs per core instead of 512) alternating the SP/ACT HWDGE rings.
    Larger sequential reads improve HBM efficiency under the pair-core
    contention that caps the shared phase at ~325 GB/s.
  - partition p of granule g holds k rows {g*1024 + 8p + j, j=0..7}; per
    sub-column j: DVE scalar_tensor_tensor (c*scale)*qb with accum_out ->
    raw score column, ACT exp, then two PE matmuls (e_t.T @ c cols) into
    per-sample PSUM rows.
  - per-sample PSUM accumulators [1, 512] x2 (PE requires matmul output
    base partition in {0, 32, 64}); denominator via ones-stationary matmul,
    DVE reciprocal, scaled ACT copies, per-row store (131 KiB total).
  - per-sample query row DMA + GPSIMD partition_broadcast (tiny).
"""

import numpy as np
from contextlib import ExitStack

import concourse.bacc as bacc
import concourse.tile as tile
from concourse import mybir
from concourse.bass_utils import run_bass_kernel_spmd

B, K, D = 256, 2048, 1024
NCORES = 8
BL = B // NCORES          # 32 samples per core
GR = 2                    # granules per sample (4 MiB DMA each)
KG = K // GR              # 1024 k-rows per granule
JJ = KG // 128            # 8 sub-columns per granule
NT = GR * JJ              # 16 e-columns per sample
SCALE = 1.0 / float(np.sqrt(D))

_cache = {}


def build_nc():
    nc = bacc.Bacc("TRN2", target_bir_lowering=False, debug=False,
                   num_devices=NCORES)
    q = nc.dram_tensor("query", [BL, D], mybir.dt.float32, kind="ExternalInput")
    c = nc.dram_tensor("concept", [BL, GR, 128, JJ * D], mybir.dt.float32r,
                       kind="ExternalInput")
    out = nc.dram_tensor("out", [BL, D], mybir.dt.float32,
                         kind="ExternalOutput")
    f32 = mybir.dt.float32
    f32r = mybir.dt.float32r

    with tile.TileContext(nc) as tc, ExitStack() as ctx:
        cpool = ctx.enter_context(tc.tile_pool(name="c", bufs=5))
        qpool = ctx.enter_context(tc.tile_pool(name="q", bufs=2))
        spool = ctx.enter_context(tc.tile_pool(name="scr", bufs=2))
        epool = ctx.enter_context(tc.tile_pool(name="e", bufs=3))
        onepool = ctx.enter_context(tc.tile_pool(name="one", bufs=1))
        opool = ctx.enter_context(tc.tile_pool(name="o", bufs=3))
        ppool = ctx.enter_context(tc.tile_pool(name="ps", bufs=2, space="PSUM"))
        dpool = ctx.enter_context(tc.tile_pool(name="dn", bufs=2, space="PSUM"))

        ones = onepool.tile([128, 1], f32)
        nc.vector.memset(ones[:], 1.0)

        for b in range(BL):
            qrow = qpool.tile([1, D], f32)
            nc.scalar.dma_start(out=qrow[:], in_=q[b : b + 1, :])
            qb = qpool.tile([128, D], f32)
            nc.gpsimd.partition_broadcast(qb[:], qrow[:])

            scols = epool.tile([128, NT], f32)
            ecols = epool.tile([128, NT], f32r)
            acc_lo = ppool.tile([1, 512], f32)
            acc_hi = ppool.tile([1, 512], f32)

            for g in range(GR):
                ct = cpool.tile([128, JJ * D], f32r)
                dma_eng = nc.sync if (b * GR + g) % 2 == 0 else nc.scalar
                dma_eng.dma_start(out=ct[:], in_=c[b, g])
                for j in range(JJ):
                    t = g * JJ + j
                    scr = spool.tile([128, D], f32)
                    nc.vector.scalar_tensor_tensor(
                        out=scr[:],
                        in0=ct[:, j * D : (j + 1) * D].bitcast(f32),
                        scalar=SCALE,
                        in1=qb[:],
                        op0=mybir.AluOpType.mult,
                        op1=mybir.AluOpType.mult,
                        accum_out=scols[:, t : t + 1],
                    )
                    nc.scalar.activation(
                        out=ecols[:, t : t + 1],
                        in_=scols[:, t : t + 1],
                        func=mybir.ActivationFunctionType.Exp,
                    )
                    e_t = ecols[:, t : t + 1]
                    nc.tensor.matmul(acc_lo[:], e_t,
                                     ct[:, j * D : j * D + 512],
                                     start=(t == 0), stop=(t == NT - 1))
                    nc.tensor.matmul(acc_hi[:], e_t,
                                     ct[:, j * D + 512 : (j + 1) * D],
                                     start=(t == 0), stop=(t == NT - 1))

            # denominator: per-partition sums of e, then reduce across
            # partitions with a ones-stationary matmul
            ered = epool.tile([128, 1], f32)
            escr = epool.tile([128, NT], f32)
            nc.scalar.activation(
                out=escr[:],
                in_=ecols[:].bitcast(f32),
                func=mybir.ActivationFunctionType.Copy,
                accum_out=ered[:],
            )
            denom = dpool.tile([1, 1], f32)
            nc.tensor.matmul(denom[:], ones[:], ered[:], start=True, stop=True)

            recip = opool.tile([1, 1], f32)
            nc.vector.reciprocal(recip[:], denom[:])
            orow = opool.tile([1, D], f32)
            nc.scalar.activation(out=orow[:, 0:512], in_=acc_lo[:],
                                 func=mybir.ActivationFunctionType.Copy,
                                 scale=recip[:])
            nc.scalar.activation(out=orow[:, 512:1024], in_=acc_hi[:],
                                 func=mybir.ActivationFunctionType.Copy,
                                 scale=recip[:])
            nc.scalar.dma_start(out=out[b : b + 1, :], in_=orow[:])

    nc.compile()
    return nc


def _run(query, concept, trace=False, trace_kwargs=None):
    if "nc" not in _cache:
        _cache["nc"] = build_nc()
    nc = _cache["nc"]
    in_maps = []
    for i in range(NCORES):
        cshard = np.ascontiguousarray(concept[i * BL : (i + 1) * BL])
        in_maps.append({
            "query": np.ascontiguousarray(query[i * BL : (i + 1) * BL]),
            "concept": cshard.reshape(BL, GR, 128, JJ * D),
        })
    res = run_bass_kernel_spmd(
        nc, in_maps, core_ids=list(range(NCORES)),
        trace=trace, **(trace_kwargs or {}),
    )
    out = np.concatenate([res.results[i]["out"] for i in range(NCORES)], axis=0)
    return out.astype(np.float32), res


def kernel(query: np.ndarray, concept: np.ndarray) -> np.ndarray:
    out, _ = _run(np.asarray(query, np.float32), np.asarray(concept, np.float32))
    return out
